# revision 11
# baseline (speedup 1.0000x reference)
"""nn_BiTransformer_42288247997027 — Trainium2 Bass kernel (fp8 DoubleRow).

Data-parallel over batch: 8 batch elements -> 8 NeuronCores, no collectives.
Per core: embedding gather (indirect DMA from the full vocab tables) + two
transformer layers. All large matmuls run in fp8 e4m3 with DoubleRow perf
mode (2 contraction rows / cycle); accumulation is fp32 in PSUM; residuals,
layernorm stats and softmax denominators stay fp32/bf16.

Scaling scheme (all powers of 2, so exact in fp32):
  - The residual stream is carried as x' = 1024*x. LN is scale-invariant,
    and both residual-add matmul outputs (o@wo, g@w2) are arranged to
    produce exactly 1024*delta in PSUM so the adds need no rescale.
  - h  = LN(x) quantized as 16*h   (fp8)
  - wq/wk/wv/w1 quantized as 64*w  (fp8), wo as 32*wo, w2 as 1024*w2
  - q,k carried as 8*q; v as 8*v; P (softmax probs) as 256*P; o as 32*o
  - gelu output unscaled in fp8
  - final output pass multiplies by 1/1024 before DMA out.
"""


import math
import sys

sys.path.insert(0, "/opt/trn_rl_repo")

import ml_dtypes
import numpy as np

import concourse.bass as bass
import concourse.mybir as mybir
import concourse.tile as tile
from concourse import bacc
from concourse.bass import IndirectOffsetOnAxis
from concourse.bass_utils import run_bass_kernel_spmd
from concourse.masks import make_identity

F32 = mybir.dt.float32
F32R = mybir.dt.float32r
F8 = mybir.dt.float8e4
BF16 = mybir.dt.bfloat16
U8 = mybir.dt.uint8
I32 = mybir.dt.int32
AF = mybir.ActivationFunctionType
ALU = mybir.AluOpType
AX = mybir.AxisListType
DR = mybir.MatmulPerfMode.DoubleRow

B, S_, D, H, DH, R, V = 8, 1024, 1024, 8, 512, 36, 32002
HD = H * DH
P = 128
T = S_
TT = T // P          # 8 token tiles
DT = D // P          # 8 feature chunks
DT2 = DT // 2        # 4 DoubleRow feature pairs
DHT = DH // P        # 4 dh chunks per head
DHT2 = DHT // 2      # 2 DoubleRow dh pairs
LN_EPS = 1e-5
SCALE = 1.0 / math.sqrt(DH)

RS = 1024.0          # residual stream carry scale
SH = 16.0            # h fp8 scale
SW = 64.0            # wq/wk/wv/w1 fp8 scale
SQK = 8.0            # q/k fp8 scale
SV = 8.0             # v fp8 scale
SP = 256.0           # P fp8 scale
SO = 32.0            # o fp8 scale
SWO = 32.0           # wo fp8 scale
SW2 = RS             # w2 fp8 scale

QK_EVAC = SQK / (SH * SW)      # 1/128
V_EVAC = SV / (SH * SW)        # 1/128
EXP_SCALE = SCALE / (SQK * SQK)
OT_EVAC = SO / (SP * SV)       # 1/64
GELU_SCALE = 1.0 / (SH * SW)   # 1/1024


def _r(ap):
    return ap.bitcast(F32R)


def _f8(ap):
    return ap.bitcast(F8)


def build_nc(n_layers=2):
    """Build + compile the per-core program. Returns compiled Bacc."""
    nc = bacc.Bacc("TRN2", target_bir_lowering=False, debug=False, num_devices=8)

    # ---------------- DRAM params ----------------
    idx_d = nc.declare_dram_parameter("idx", [P, TT], I32, isOutput=False)
    img_d = nc.declare_dram_parameter("img", [R, D], F32, isOutput=False)
    emb_d = nc.declare_dram_parameter("emb", [V, D], F32, isOutput=False)
    i2v_d = nc.declare_dram_parameter("i2v", [V, R], F32, isOutput=False)
    Ws = []
    for l in range(n_layers):
        w = {}
        w["wq"] = nc.declare_dram_parameter(f"wq{l}", [H * DT2, P, 2, DH], U8, isOutput=False)
        w["wk"] = nc.declare_dram_parameter(f"wk{l}", [H * DT2, P, 2, DH], U8, isOutput=False)
        w["wv"] = nc.declare_dram_parameter(f"wv{l}", [H * DT2, P, 2, DH], U8, isOutput=False)
        w["wo"] = nc.declare_dram_parameter(f"wo{l}", [H * DHT2, P, 2, D], U8, isOutput=False)
        w["w1"] = nc.declare_dram_parameter(f"w1{l}", [DT2, P, 2, D], U8, isOutput=False)
        w["w2"] = nc.declare_dram_parameter(f"w2{l}", [DT2, P, 2, D], U8, isOutput=False)
        Ws.append(w)
    out_d = nc.declare_dram_parameter("out", [T, D], F32, isOutput=True)

    from contextlib import ExitStack
    with tile.TileContext(nc) as tc, ExitStack() as ctx:
        consts = ctx.enter_context(tc.tile_pool(name="consts", bufs=1))
        xpool = ctx.enter_context(tc.tile_pool(name="xpool", bufs=TT))
        big = ctx.enter_context(tc.tile_pool(name="big", bufs=2))
        qko_p = ctx.enter_context(tc.tile_pool(name="qko", bufs=2))
        vpool = ctx.enter_context(tc.tile_pool(name="vp", bufs=8))
        hpool = ctx.enter_context(tc.tile_pool(name="hp", bufs=2))
        ppool = ctx.enter_context(tc.tile_pool(name="pp", bufs=4))
        ptp = ctx.enter_context(tc.tile_pool(name="ptp", bufs=2))
        wp5 = ctx.enter_context(tc.tile_pool(name="wp5", bufs=8))
        wp10 = ctx.enter_context(tc.tile_pool(name="wp10", bufs=4))
        small = ctx.enter_context(tc.tile_pool(name="small", bufs=2))
        ps = ctx.enter_context(tc.tile_pool(name="ps", bufs=4, space="PSUM"))

        def psum_tile(name):
            return ps.tile([P, 1024], F32, tag="ps", name=name)

        ident_tmp = hpool.tile([P, P], F32, tag="ident", name="ident_tmp", bufs=1)
        make_identity(nc, ident_tmp)
        identr = consts.tile([P, P], F32R)
        nc.vector.tensor_copy(identr, ident_tmp)
        identb = consts.tile([P, P], BF16)
        nc.vector.tensor_copy(identb, ident_tmp)
        eps_t = consts.tile([P, 1], F32)
        nc.vector.memset(eps_t, LN_EPS / 256.0)
        idx_sb = consts.tile([P, TT], I32)
        nc.sync.dma_start(idx_sb, idx_d.ap())
        img_sb = consts.tile([R, D], F32R)
        nc.sync.dma_start(img_sb, _r(img_d.ap()))

        # ---------------- embedding ----------------
        # hT(layer 0) LN+transpose is interleaved per tile right after each
        # x tile completes, so the PE has transposes/matmuls during gathers.
        x_tiles = []
        for t in range(TT):
            xt = xpool.tile([P, D], F32, tag="x", name=f"x{t}")
            x_tiles.append(xt)
        hT = big.tile([P, DT, T], F8, tag="hT", name="hT0")
        vids_all = small.tile([P, TT, R], F32R, tag="vidsall", bufs=1)
        for t in range(TT):
            nc.gpsimd.indirect_dma_start(
                out=vids_all[:, t, :], out_offset=None, in_=_r(i2v_d.ap()),
                in_offset=IndirectOffsetOnAxis(ap=idx_sb[:, t:t + 1], axis=0))
        m01s = []
        for t in range(TT):
            vids = vids_all[:, t, :]
            vsum = small.tile([P, 1], F32, tag="vsum")
            nc.vector.reduce_sum(vsum, vids, axis=AX.X)
            m01 = small.tile([P, 1], F32, tag=f"m01_{t}", bufs=1)
            # 1024 where the token is textual (no visual row), 0 otherwise;
            # folds the residual-stream carry scale into the embedding mask.
            nc.vector.tensor_scalar(m01, vsum, 0.0, RS, op0=ALU.is_equal, op1=ALU.mult)
            m01s.append(m01)
            vt_ps = psum_tile(f"vtp{t}")
            nc.tensor.transpose(vt_ps[:R, :P].bitcast(F32R), vids, identr)
            vt_sb = small.tile([R, P], F32R, tag="vt", bufs=2)
            nc.vector.tensor_copy(vt_sb, vt_ps[:R, :P].bitcast(F32R))
            ve_ps = psum_tile(f"vep{t}")
            for nh in range(2):
                nc.tensor.matmul(ve_ps[:, nh * 512:(nh + 1) * 512], lhsT=vt_sb,
                                 rhs=img_sb[:, nh * 512:(nh + 1) * 512],
                                 start=True, stop=True)
            xt = x_tiles[t]
            nc.gpsimd.indirect_dma_start(
                out=xt[:, :], out_offset=None, in_=emb_d.ap(),
                in_offset=IndirectOffsetOnAxis(ap=idx_sb[:, t:t + 1], axis=0))
            nc.vector.tensor_scalar_mul(xt[:, :], xt[:, :], m01s[t])
            nc.vector.tensor_add(xt[:, :], xt[:, :], ve_ps[:, :])
            _ln_transpose(nc, tc, hpool, small, psum_tile,
                          x_tiles[t], eps_t, identb, hT, t, f"h0_{t}")

        # ---------------- transformer layers ----------------
        for l in range(n_layers):
            w = Ws[l]

            # ---- heads (hT for this layer was produced by the previous
            # layer's FFN evacuation loop, or the embedding loop for l=0)
            for hh in range(H):
                # q^T and k^T : [P, DHT, T] fp8, weights stationary.
                # m-outer, j-inner accumulation so each qT/kT chunk evacuates
                # as soon as its 4 matmuls are done (feeds the softmax chain
                # early); the 4 weight DMAs are hoisted ahead of the matmuls.
                qT = qko_p.tile([P, DHT, T], F8, tag="qko", name=f"qT{l}_{hh}")
                kT = qko_p.tile([P, DHT, T], F8, tag="qko", name=f"kT{l}_{hh}")
                for wd, dst, bname in ((w["wq"], qT, "bq"), (w["wk"], kT, "bk")):
                    wts = []
                    for j in range(DT2):
                        wt = wp5.tile([P, 2, DH], F8, tag="w5",
                                      name=f"w5_{l}_{hh}_{bname}{j}")
                        nc.sync.dma_start(wt, _f8(wd.ap()[hh * DT2 + j]))
                        wts.append(wt)
                    pss = [psum_tile(f"pj{l}_{hh}_{bname}{m}") for m in range(DHT)]
                    for m in range(DHT):
                        for j in range(DT2):
                            for nh in range(2):
                                nc.tensor.matmul(
                                    pss[m][:, nh * 512:(nh + 1) * 512],
                                    lhsT=wts[j][:, :, m * P:(m + 1) * P],
                                    rhs=hT[:, 2 * j:2 * j + 2, nh * 512:(nh + 1) * 512],
                                    start=(j == 0), stop=(j == DT2 - 1),
                                    perf_mode=DR)
                        nc.scalar.activation(dst[:, m, :], pss[m][:, :], AF.Identity,
                                             bias=0.0, scale=QK_EVAC)

                ptiles = [ptp.tile([P, TT, 512], F8, tag="pt", name=f"pt{l}_{hh}_{hf}")
                          for hf in range(2)]
                pes = {}

                def softmax_tile(qi):
                    sps = psum_tile(f"s{l}_{hh}_{qi}")
                    for dd in range(DHT2):
                        for nh in range(2):
                            nc.tensor.matmul(
                                sps[:, nh * 512:(nh + 1) * 512],
                                lhsT=qT[:, 2 * dd:2 * dd + 2, qi * P:(qi + 1) * P],
                                rhs=kT[:, 2 * dd:2 * dd + 2, nh * 512:(nh + 1) * 512],
                                start=(dd == 0), stop=(dd == DHT2 - 1),
                                perf_mode=DR)
                    # unshifted softmax: e = exp(S*c); denom = rowmax(e)+rowsum(e)
                    pe = ppool.tile([P, T], BF16, tag="P", name=f"P{l}_{hh}_{qi}", bufs=2)
                    ssum = small.tile([P, 1], F32, tag="ssum")
                    nc.scalar.activation(pe[:, :], sps[:, :], AF.Exp,
                                         bias=0.0, scale=EXP_SCALE, accum_out=ssum)
                    smax = small.tile([P, 1], F32, tag="smax")
                    nc.vector.reduce_max(smax, pe[:, :], axis=AX.X)
                    nc.vector.tensor_tensor(smax, smax, ssum, op=ALU.add)
                    rdenom = small.tile([P, 1], F32, tag="rden")
                    nc.vector.reciprocal(rdenom, smax)
                    pe2 = ppool.tile([P, T], BF16, tag="P8", name=f"P8{l}_{hh}_{qi}", bufs=4)
                    nc.vector.tensor_scalar(pe2, pe, rdenom, SP, op0=ALU.mult, op1=ALU.mult)
                    pes[qi] = pe2

                def transpose_tile(qi):
                    # all 8 [128,128] bf16 transposes of P(qi) land in one psum
                    # tile, evacuated (and converted to fp8) by ONE strided
                    # copy on the Scalar engine.
                    pe2 = pes.pop(qi)
                    ptile = ptiles[qi // 4]
                    tp = psum_tile(f"ptp{l}_{hh}_{qi}")
                    tpr = tp[:, :512].bitcast(BF16)
                    for tk in range(TT):
                        nc.tensor.transpose(tpr[:, tk * P:(tk + 1) * P],
                                            pe2[:, tk * P:(tk + 1) * P], identb)
                    nc.scalar.copy(
                        ptile[:, :, (qi % 4) * P:(qi % 4 + 1) * P],
                        tpr.rearrange("p (tk c) -> p tk c", c=P))

                def pv_half(half, oT):
                    ptile = ptiles[half]
                    # PV for this half: oT[:, m, half*512 : +512]
                    for m in range(DHT):
                        ops_ = psum_tile(f"o{l}_{hh}_{half}_{m}")
                        for j in range(TT // 2):
                            nc.tensor.matmul(
                                ops_[:, :512],
                                lhsT=v4[j][:, :, m * P:(m + 1) * P],
                                rhs=ptile[:, 2 * j:2 * j + 2, :],
                                start=(j == 0), stop=(j == TT // 2 - 1),
                                perf_mode=DR)
                        if m % 2 == 0:
                            nc.vector.tensor_scalar(
                                oT[:, m, half * 512:(half + 1) * 512],
                                ops_[:, :512], OT_EVAC, None, op0=ALU.mult)
                        else:
                            nc.scalar.activation(
                                oT[:, m, half * 512:(half + 1) * 512],
                                ops_[:, :512], AF.Identity, bias=0.0, scale=OT_EVAC)

                # S(q0..q2) first so their ACT/DVE softmax chains run behind
                # the v matmuls; then v; then the remaining S tiles software-
                # pipelined with the P^T transposes (lag 3).
                for qi in range(3):
                    softmax_tile(qi)

                # v token-major: 4 tiles [P(tok), 2, DH] fp8; hT stationary;
                # t-outer, j-inner so each v evacuation fires early.
                v4 = [vpool.tile([P, 2, DH], F8, tag="v", name=f"v{l}_{hh}_{j}")
                      for j in range(TT // 2)]
                vwts = []
                for j in range(DT2):
                    wt = wp5.tile([P, 2, DH], F8, tag="w5", name=f"w5v_{l}_{hh}_{j}")
                    nc.sync.dma_start(wt, _f8(w["wv"].ap()[hh * DT2 + j]))
                    vwts.append(wt)
                pvs = [psum_tile(f"pv{l}_{hh}_{j}") for j in range(TT // 2)]
                for t in range(TT):
                    half = (t % 2) * 512
                    for j in range(DT2):
                        nc.tensor.matmul(
                            pvs[t // 2][:, half:half + DH],
                            lhsT=hT[:, 2 * j:2 * j + 2, t * P:(t + 1) * P],
                            rhs=vwts[j][:, :, :],
                            start=(j == 0), stop=(j == DT2 - 1),
                            perf_mode=DR)
                    nc.scalar.activation(v4[t // 2][:, t % 2, :],
                                         pvs[t // 2][:, half:half + DH],
                                         AF.Identity, bias=0.0, scale=V_EVAC)

                for qi in range(3, TT):
                    softmax_tile(qi)
                    transpose_tile(qi - 3)
                # PV(half0) here covers the softmax-chain tail of q-tiles 5..7
                transpose_tile(TT - 3)
                oT = qko_p.tile([P, DHT, T], F8, tag="qko", name=f"oT{l}_{hh}")
                pv_half(0, oT)
                transpose_tile(TT - 2)
                transpose_tile(TT - 1)
                pv_half(1, oT)

                # o @ wo -> token-major x update, t-outer in two waves of 4
                # token tiles ([128,1024] full-D psum per tile), so each x
                # update (and, on the last head, its LN2 + fT transpose) fires
                # while the next wave's matmuls still feed the PE.
                last = (hh == H - 1)
                if last:
                    fT = big.tile([P, DT, T], F8, tag="hT", name=f"fT{l}")
                owts = {}
                for dh2 in range(2):
                    for j in range(DHT2):
                        wt = wp5.tile([P, 2, 512], F8, tag="w5",
                                      name=f"wo_{l}_{hh}_{dh2}_{j}")
                        nc.sync.dma_start(
                            wt, _f8(w["wo"].ap()[hh * DHT2 + j, :, :,
                                                 dh2 * 512:dh2 * 512 + 512]))
                        owts[(dh2, j)] = wt
                for t in range(TT):
                    px = psum_tile(f"px{l}_{hh}_{t}")
                    for dh2 in range(2):
                        for j in range(DHT2):
                            nc.tensor.matmul(
                                px[:, dh2 * 512:dh2 * 512 + 512],
                                lhsT=oT[:, 2 * j:2 * j + 2, t * P:(t + 1) * P],
                                rhs=owts[(dh2, j)][:, :, :],
                                start=(j == 0), stop=(j == DHT2 - 1),
                                perf_mode=DR)
                    nc.vector.tensor_add(x_tiles[t][:, :], x_tiles[t][:, :], px[:, :])
                    if last:
                        _ln_transpose(nc, tc, hpool, small, psum_tile,
                                      x_tiles[t], eps_t, identb, fT, t, f"f{l}_{t}")

            # ---- FFN. After the f2 update of each token tile, the next
            # layer's LN1 + hT transpose (or the final output scale + DMA)
            # runs immediately, overlapped with the remaining f2 matmuls.
            if l + 1 < n_layers:
                hT = big.tile([P, DT, T], F8, tag="hT", name=f"hT{l + 1}")
            for half in range(2):
                toff = half * 512
                # f1 half: feature-major [D, T/2] fp8; w1 stationary;
                # dm-outer, j-inner so each gelu evacuation fires early.
                f1g = ptp.tile([P, DT, 512], F8, tag="pt", name=f"f1g{l}_{half}")
                w1ts = []
                for j in range(DT2):
                    wt = wp10.tile([P, 2, D], F8, tag="w10", name=f"w1_{l}_{half}_{j}")
                    nc.sync.dma_start(wt, _f8(w["w1"].ap()[j]))
                    w1ts.append(wt)
                pfs = [psum_tile(f"pf{l}_{half}_{j}") for j in range(4)]
                for dm in range(DT):
                    pslc = pfs[dm // 2][:, (dm % 2) * 512:(dm % 2) * 512 + 512]
                    for j in range(DT2):
                        nc.tensor.matmul(
                            pslc,
                            lhsT=w1ts[j][:, :, dm * P:(dm + 1) * P],
                            rhs=fT[:, 2 * j:2 * j + 2, toff:toff + 512],
                            start=(j == 0), stop=(j == DT2 - 1),
                            perf_mode=DR)
                    nc.scalar.activation(f1g[:, dm, :], pslc, AF.Gelu,
                                         bias=0.0, scale=GELU_SCALE)
                # f2 half: token-major; f1g stationary; jj-outer, j-inner
                w2ts = []
                for j in range(DT2):
                    wt = wp10.tile([P, 2, D], F8, tag="w10", name=f"w2_{l}_{half}_{j}")
                    nc.sync.dma_start(wt, _f8(w["w2"].ap()[j]))
                    w2ts.append(wt)
                pxs = [psum_tile(f"pg{l}_{half}_{j}") for j in range(4)]
                for jj in range(4):
                    tq = half * 4 + jj
                    for j in range(DT2):
                        for nh in range(2):
                            nc.tensor.matmul(
                                pxs[jj][:, nh * 512:(nh + 1) * 512],
                                lhsT=f1g[:, 2 * j:2 * j + 2, jj * P:(jj + 1) * P],
                                rhs=w2ts[j][:, :, nh * 512:(nh + 1) * 512],
                                start=(j == 0), stop=(j == DT2 - 1),
                                perf_mode=DR)
                    nc.vector.tensor_add(x_tiles[tq][:, :], x_tiles[tq][:, :], pxs[jj][:, :])
                    if l + 1 < n_layers:
                        _ln_transpose(nc, tc, hpool, small, psum_tile,
                                      x_tiles[tq], eps_t, identb, hT, tq,
                                      f"h{l + 1}_{tq}")
                    else:
                        nc.scalar.activation(x_tiles[tq][:, :], x_tiles[tq][:, :],
                                             AF.Identity, bias=0.0, scale=1.0 / RS)
                        nc.sync.dma_start(out_d.ap()[tq * P:(tq + 1) * P, :],
                                          x_tiles[tq][:, :])

    nc.compile()
    return nc


def _ln_transpose(nc, tc, hpool, small, psum_tile, x_t, eps_t, identb, dstT, t, name):
    """LayerNorm one token tile (output scaled by SH, bf16), transpose it
    into dstT[:, :, t*128:+128] (fp8 conversion at the evacuation copy)."""
    h2 = _ln_tile(nc, tc, hpool, small, x_t, eps_t, name)
    tp = psum_tile(f"tp_{name}")
    tpr = tp[:, :512].bitcast(BF16)
    for d in range(DT):
        nc.tensor.transpose(tpr[:, d * P:(d + 1) * P],
                            h2[:, d * P:(d + 1) * P], identb)
    nc.scalar.copy(dstT[:, :, t * P:(t + 1) * P],
                   tpr.rearrange("p (d c) -> p d c", c=P))


def _ln_tile(nc, tc, hpool, small, x_t, eps_t, name):
    """LayerNorm core SH*(x-mean)*rstd of one [128, D] tile -> bf16 h tile.
    The Sqrt activation computes sqrt(var/256 + eps/256) = sqrt(var+eps)/16,
    so the reciprocal directly yields 16*rstd (= SH fold, no extra op)."""
    stats = small.tile([P, 2, 6], F32, tag="bnst", name=f"st_{name}")
    for g in range(2):
        nc.vector.bn_stats(stats[:, g, :], x_t[:, g * 512:(g + 1) * 512])
    mv = small.tile([P, 2], F32, tag="mv", name=f"mv_{name}")
    nc.vector.bn_aggr(mv, stats)
    std = small.tile([P, 1], F32, tag="std", name=f"sd_{name}")
    nc.scalar.activation(std, mv[:, 1:2], AF.Sqrt, bias=eps_t, scale=1.0 / 256.0)
    rstd = small.tile([P, 1], F32, tag="rstd", name=f"rs_{name}")
    nc.vector.reciprocal(rstd, std)
    h2 = hpool.tile([P, D], BF16, tag="h", name=f"h_{name}")
    nc.vector.tensor_scalar(h2, x_t, mv[:, 0:1], rstd, op0=ALU.subtract, op1=ALU.mult)
    return h2


# ---------------- host side ----------------

def _q8(w, s):
    """Quantize w*s to fp8 e4m3, return as uint8 bytes."""
    return np.asarray(np.asarray(w, np.float32) * s).astype(ml_dtypes.float8_e4m3).view(np.uint8)


def prep_inputs(inputs, n_layers=2):
    """Fold LN gains into weights, quantize to fp8, rearrange for the device.
    Returns (shared_map, per_core_list, use_biases=False)."""
    f = np.float32
    pre_words = np.asarray(inputs["pre_words"])
    img = np.asarray(inputs["img_features"], dtype=f)
    emb = np.ascontiguousarray(np.asarray(inputs["exp_embed"], dtype=f))
    i2v = np.ascontiguousarray(np.asarray(inputs["id2vis"], dtype=f))

    shared = {"emb": emb, "i2v": i2v}
    for l in range(n_layers):
        g1 = np.asarray(inputs["ln1_g"][l], dtype=f)
        b1l = np.asarray(inputs["ln1_b"][l], dtype=f)
        g2 = np.asarray(inputs["ln2_g"][l], dtype=f)
        b2l = np.asarray(inputs["ln2_b"][l], dtype=f)
        wq = np.asarray(inputs["wq"][l], dtype=f) * g1[:, None]
        wk = np.asarray(inputs["wk"][l], dtype=f) * g1[:, None]
        wv = np.asarray(inputs["wv"][l], dtype=f) * g1[:, None]
        wo = np.asarray(inputs["wo"][l], dtype=f)
        w1 = np.asarray(inputs["w1"][l], dtype=f) * g2[:, None]
        w2 = np.asarray(inputs["w2"][l], dtype=f)
        # effective biases must be zero for this kernel (they are, by
        # construction of setup_inputs: zero biases and zero LN betas)
        bq = b1l @ np.asarray(inputs["wq"][l], dtype=f) + np.asarray(inputs["bq"][l], dtype=f)
        bk = b1l @ np.asarray(inputs["wk"][l], dtype=f) + np.asarray(inputs["bk"][l], dtype=f)
        bv = b1l @ np.asarray(inputs["wv"][l], dtype=f) + np.asarray(inputs["bv"][l], dtype=f)
        b1 = b2l @ np.asarray(inputs["w1"][l], dtype=f) + np.asarray(inputs["b1"][l], dtype=f)
        for a in (bq, bk, bv, b1, np.asarray(inputs["bo"][l]), np.asarray(inputs["b2"][l])):
            if np.any(a != 0):
                raise NotImplementedError("nonzero effective biases unsupported")
        # [D, HD] -> [H*DT2, P, 2, DH]; k = j*256 + kp*128 + p
        shared[f"wq{l}"] = np.ascontiguousarray(
            _q8(wq, SW).reshape(DT2, 2, P, H, DH).transpose(3, 0, 2, 1, 4)
            .reshape(H * DT2, P, 2, DH))
        shared[f"wk{l}"] = np.ascontiguousarray(
            _q8(wk, SW).reshape(DT2, 2, P, H, DH).transpose(3, 0, 2, 1, 4)
            .reshape(H * DT2, P, 2, DH))
        shared[f"wv{l}"] = np.ascontiguousarray(
            _q8(wv, SW).reshape(DT2, 2, P, H, DH).transpose(3, 0, 2, 1, 4)
            .reshape(H * DT2, P, 2, DH))
        # [HD, D] -> [H*DHT2, P, 2, D]
        shared[f"wo{l}"] = np.ascontiguousarray(
            _q8(wo, SWO).reshape(H, DHT2, 2, P, D).transpose(0, 1, 3, 2, 4)
            .reshape(H * DHT2, P, 2, D))
        # [D, D] -> [DT2, P, 2, D]
        shared[f"w1{l}"] = np.ascontiguousarray(
            _q8(w1, SW).reshape(DT2, 2, P, D).transpose(0, 2, 1, 3))
        shared[f"w2{l}"] = np.ascontiguousarray(
            _q8(w2, SW2).reshape(DT2, 2, P, D).transpose(0, 2, 1, 3))

    per_core = []
    for b in range(B):
        idx = np.ascontiguousarray(
            pre_words[b].astype(np.int32).reshape(TT, P).T)
        per_core.append({"idx": idx,
                         "img": np.ascontiguousarray(img[b] * RS)})
    return shared, per_core, False


def make_in_maps(shared, per_core, use_biases=False, n_layers=2):
    keys = ["emb", "i2v"]
    for l in range(n_layers):
        keys += [f"wq{l}", f"wk{l}", f"wv{l}", f"wo{l}", f"w1{l}", f"w2{l}"]
    maps = []
    for b in range(B):
        m = {k: shared[k] for k in keys}
        m.update(per_core[b])
        maps.append(m)
    return maps


# ---------------- public entry point ----------------

_CACHE = {}


def _get_nc(n_layers=2, use_biases=False):
    key = n_layers
    if key not in _CACHE:
        _CACHE[key] = build_nc(n_layers=n_layers)
    return _CACHE[key]


def kernel(**inputs):
    shared, per_core, use_biases = prep_inputs(inputs, n_layers=2)
    nc = _get_nc(2, use_biases)
    in_maps = make_in_maps(shared, per_core, use_biases, n_layers=2)
    res = run_bass_kernel_spmd(nc, in_maps, list(range(8)))
    out = np.stack([res.results[i]["out"] for i in range(8)]).astype(np.float32)
    return out


# revision 15
# speedup vs baseline: 1.3063x; 1.3063x over previous
"""nn_BiTransformer_42288247997027 — Trainium2 Bass kernel (fp8 DoubleRow).

Data-parallel over batch: 8 batch elements -> 8 NeuronCores, no collectives.
Per core: embedding gather (indirect DMA from the full vocab tables) + two
transformer layers. All large matmuls run in fp8 e4m3 with DoubleRow perf
mode (2 contraction rows / cycle); accumulation is fp32 in PSUM; residuals,
layernorm stats and softmax denominators stay fp32/bf16.

Scaling scheme (all powers of 2, so exact in fp32):
  - The residual stream is carried as x' = 1024*x. LN is scale-invariant,
    and both residual-add matmul outputs (o@wo, g@w2) are arranged to
    produce exactly 1024*delta in PSUM so the adds need no rescale.
  - h  = LN(x) quantized as 16*h   (fp8)
  - wq/wk/wv/w1 quantized as 64*w  (fp8), wo as 32*wo, w2 as 1024*w2
  - q,k carried as 8*q; v as 8*v; P (softmax probs) as 256*P; o as 32*o
  - gelu output unscaled in fp8
  - final output pass multiplies by 1/1024 before DMA out.
"""


import math
import sys

sys.path.insert(0, "/opt/trn_rl_repo")

import ml_dtypes
import numpy as np

import concourse.bass as bass
import concourse.mybir as mybir
import concourse.tile as tile
from concourse import bacc
from concourse.bass import IndirectOffsetOnAxis
from concourse.bass_utils import run_bass_kernel_spmd
from concourse.masks import make_identity

F32 = mybir.dt.float32
F32R = mybir.dt.float32r
F8 = mybir.dt.float8e4
BF16 = mybir.dt.bfloat16
U8 = mybir.dt.uint8
I32 = mybir.dt.int32
AF = mybir.ActivationFunctionType
ALU = mybir.AluOpType
AX = mybir.AxisListType
DR = mybir.MatmulPerfMode.DoubleRow

B, S_, D, H, DH, R, V = 8, 1024, 1024, 8, 512, 36, 32002
HD = H * DH
P = 128
T = S_
TT = T // P          # 8 token tiles
DT = D // P          # 8 feature chunks
DT2 = DT // 2        # 4 DoubleRow feature pairs
DHT = DH // P        # 4 dh chunks per head
DHT2 = DHT // 2      # 2 DoubleRow dh pairs
LN_EPS = 1e-5
SCALE = 1.0 / math.sqrt(DH)

RS = 1024.0          # residual stream carry scale
SH = 16.0            # h fp8 scale
SW = 64.0            # wq/wk/wv/w1 fp8 scale
SQK = 8.0            # q/k fp8 scale
SV = 8.0             # v fp8 scale
SP = 256.0           # P fp8 scale
SO = 32.0            # o fp8 scale
SWO = 32.0           # wo fp8 scale
SW2 = RS             # w2 fp8 scale

QK_EVAC = SQK / (SH * SW)      # 1/128
V_EVAC = SV / (SH * SW)        # 1/128
EXP_SCALE = SCALE / (SQK * SQK)
OT_EVAC = SO / (SP * SV)       # 1/64
GELU_SCALE = 1.0 / (SH * SW)   # 1/1024


def _r(ap):
    return ap.bitcast(F32R)


def _f8(ap):
    return ap.bitcast(F8)


def build_nc(n_layers=2):
    """Build + compile the per-core program. Returns compiled Bacc."""
    nc = bacc.Bacc("TRN2", target_bir_lowering=False, debug=False, num_devices=8)

    # ---------------- DRAM params ----------------
    idx_d = nc.declare_dram_parameter("idx", [P, TT], I32, isOutput=False)
    img_d = nc.declare_dram_parameter("img", [R, D], F32, isOutput=False)
    emb_d = nc.declare_dram_parameter("emb", [V, D], F32, isOutput=False)
    i2v_d = nc.declare_dram_parameter("i2v", [V, R], F32, isOutput=False)
    Ws = []
    for l in range(n_layers):
        w = {}
        w["wq"] = nc.declare_dram_parameter(f"wq{l}", [H * DT2, P, 2, DH], U8, isOutput=False)
        w["wk"] = nc.declare_dram_parameter(f"wk{l}", [H * DT2, P, 2, DH], U8, isOutput=False)
        w["wv"] = nc.declare_dram_parameter(f"wv{l}", [H * DT2, P, 2, DH], U8, isOutput=False)
        w["wo"] = nc.declare_dram_parameter(f"wo{l}", [H * DHT2, P, 2, D], U8, isOutput=False)
        w["w1"] = nc.declare_dram_parameter(f"w1{l}", [DT2, P, 2, D], U8, isOutput=False)
        w["w2"] = nc.declare_dram_parameter(f"w2{l}", [DT2, P, 2, D], U8, isOutput=False)
        Ws.append(w)
    out_d = nc.declare_dram_parameter("out", [T, D], F32, isOutput=True)

    from contextlib import ExitStack
    with tile.TileContext(nc) as tc, ExitStack() as ctx:
        consts = ctx.enter_context(tc.tile_pool(name="consts", bufs=1))
        xpool = ctx.enter_context(tc.tile_pool(name="xpool", bufs=TT))
        big = ctx.enter_context(tc.tile_pool(name="big", bufs=2))
        qko_p = ctx.enter_context(tc.tile_pool(name="qko", bufs=2))
        vpool = ctx.enter_context(tc.tile_pool(name="vp", bufs=8))
        hpool = ctx.enter_context(tc.tile_pool(name="hp", bufs=2))
        ppool = ctx.enter_context(tc.tile_pool(name="pp", bufs=4))
        ptp = ctx.enter_context(tc.tile_pool(name="ptp", bufs=2))
        wp5 = ctx.enter_context(tc.tile_pool(name="wp5", bufs=8))
        wp10 = ctx.enter_context(tc.tile_pool(name="wp10", bufs=4))
        small = ctx.enter_context(tc.tile_pool(name="small", bufs=2))
        ps = ctx.enter_context(tc.tile_pool(name="ps", bufs=4, space="PSUM"))

        def psum_tile(name):
            return ps.tile([P, 1024], F32, tag="ps", name=name)

        ident_tmp = hpool.tile([P, P], F32, tag="ident", name="ident_tmp", bufs=1)
        make_identity(nc, ident_tmp)
        identr = consts.tile([P, P], F32R)
        nc.vector.tensor_copy(identr, ident_tmp)
        identb = consts.tile([P, P], BF16)
        nc.vector.tensor_copy(identb, ident_tmp)
        eps_t = consts.tile([P, 1], F32)
        nc.vector.memset(eps_t, LN_EPS / 256.0)
        idx_sb = consts.tile([P, TT], I32)
        nc.sync.dma_start(idx_sb, idx_d.ap())
        img_sb = consts.tile([R, D], F32R)
        nc.sync.dma_start(img_sb, _r(img_d.ap()))

        # ---------------- embedding ----------------
        x_tiles = []
        for t in range(TT):
            xt = xpool.tile([P, D], F32, tag="x", name=f"x{t}")
            x_tiles.append(xt)
        vids_all = small.tile([P, TT, R], F32R, tag="vidsall", bufs=1)
        for t in range(TT):
            nc.gpsimd.indirect_dma_start(
                out=vids_all[:, t, :], out_offset=None, in_=_r(i2v_d.ap()),
                in_offset=IndirectOffsetOnAxis(ap=idx_sb[:, t:t + 1], axis=0))
        m01s = []
        for t in range(TT):
            vids = vids_all[:, t, :]
            vsum = small.tile([P, 1], F32, tag="vsum")
            nc.vector.reduce_sum(vsum, vids, axis=AX.X)
            m01 = small.tile([P, 1], F32, tag=f"m01_{t}", bufs=1)
            # 1024 where the token is textual (no visual row), 0 otherwise;
            # folds the residual-stream carry scale into the embedding mask.
            nc.vector.tensor_scalar(m01, vsum, 0.0, RS, op0=ALU.is_equal, op1=ALU.mult)
            m01s.append(m01)
            vt_ps = psum_tile(f"vtp{t}")
            nc.tensor.transpose(vt_ps[:R, :P].bitcast(F32R), vids, identr)
            vt_sb = small.tile([R, P], F32R, tag="vt", bufs=2)
            nc.vector.tensor_copy(vt_sb, vt_ps[:R, :P].bitcast(F32R))
            ve_ps = psum_tile(f"vep{t}")
            for nh in range(2):
                nc.tensor.matmul(ve_ps[:, nh * 512:(nh + 1) * 512], lhsT=vt_sb,
                                 rhs=img_sb[:, nh * 512:(nh + 1) * 512],
                                 start=True, stop=True)
            xt = x_tiles[t]
            nc.gpsimd.indirect_dma_start(
                out=xt[:, :], out_offset=None, in_=emb_d.ap(),
                in_offset=IndirectOffsetOnAxis(ap=idx_sb[:, t:t + 1], axis=0))
            nc.vector.tensor_scalar_mul(xt[:, :], xt[:, :], m01s[t])
            nc.vector.tensor_add(xt[:, :], xt[:, :], ve_ps[:, :])

        # ---------------- transformer layers ----------------
        for l in range(n_layers):
            w = Ws[l]

            # ---- LN1 -> hT for l=0 only; later layers get hT from the
            # previous layer's FFN evacuation loop.
            if l == 0:
                hT = big.tile([P, DT, T], F8, tag="hT", name="hT0")
                for t in range(TT):
                    _ln_transpose(nc, tc, hpool, small, psum_tile,
                                  x_tiles[t], eps_t, identb, hT, t, f"h0_{t}")

            # ---- heads
            for hh in range(H):
                # q^T and k^T : [P, DHT, T] fp8, weights stationary.
                # m-outer, j-inner accumulation so each qT/kT chunk evacuates
                # as soon as its 4 matmuls are done (feeds the softmax chain
                # early); the 4 weight DMAs are hoisted ahead of the matmuls.
                qT = qko_p.tile([P, DHT, T], F8, tag="qko", name=f"qT{l}_{hh}")
                kT = qko_p.tile([P, DHT, T], F8, tag="qko", name=f"kT{l}_{hh}")
                for wd, dst, bname in ((w["wq"], qT, "bq"), (w["wk"], kT, "bk")):
                    wts = []
                    for j in range(DT2):
                        wt = wp5.tile([P, 2, DH], F8, tag="w5",
                                      name=f"w5_{l}_{hh}_{bname}{j}")
                        nc.sync.dma_start(wt, _f8(wd.ap()[hh * DT2 + j]))
                        wts.append(wt)
                    pss = [psum_tile(f"pj{l}_{hh}_{bname}{m}") for m in range(DHT)]
                    for m in range(DHT):
                        for j in range(DT2):
                            for nh in range(2):
                                nc.tensor.matmul(
                                    pss[m][:, nh * 512:(nh + 1) * 512],
                                    lhsT=wts[j][:, :, m * P:(m + 1) * P],
                                    rhs=hT[:, 2 * j:2 * j + 2, nh * 512:(nh + 1) * 512],
                                    start=(j == 0), stop=(j == DT2 - 1),
                                    perf_mode=DR)
                        nc.scalar.activation(dst[:, m, :], pss[m][:, :], AF.Identity,
                                             bias=0.0, scale=QK_EVAC)

                ptiles = [ptp.tile([P, TT, 512], F8, tag="pt", name=f"pt{l}_{hh}_{hf}")
                          for hf in range(2)]
                pes = {}

                def softmax_tile(qi):
                    sps = psum_tile(f"s{l}_{hh}_{qi}")
                    for dd in range(DHT2):
                        for nh in range(2):
                            nc.tensor.matmul(
                                sps[:, nh * 512:(nh + 1) * 512],
                                lhsT=qT[:, 2 * dd:2 * dd + 2, qi * P:(qi + 1) * P],
                                rhs=kT[:, 2 * dd:2 * dd + 2, nh * 512:(nh + 1) * 512],
                                start=(dd == 0), stop=(dd == DHT2 - 1),
                                perf_mode=DR)
                    # unshifted softmax: e = exp(S*c); denom = rowmax(e)+rowsum(e)
                    pe = ppool.tile([P, T], BF16, tag="P", name=f"P{l}_{hh}_{qi}", bufs=2)
                    ssum = small.tile([P, 1], F32, tag="ssum")
                    nc.scalar.activation(pe[:, :], sps[:, :], AF.Exp,
                                         bias=0.0, scale=EXP_SCALE, accum_out=ssum)
                    smax = small.tile([P, 1], F32, tag="smax")
                    nc.vector.reduce_max(smax, pe[:, :], axis=AX.X)
                    nc.vector.tensor_tensor(smax, smax, ssum, op=ALU.add)
                    rdenom = small.tile([P, 1], F32, tag="rden")
                    nc.vector.reciprocal(rdenom, smax)
                    pe2 = ppool.tile([P, T], BF16, tag="P8", name=f"P8{l}_{hh}_{qi}", bufs=4)
                    nc.vector.tensor_scalar(pe2, pe, rdenom, SP, op0=ALU.mult, op1=ALU.mult)
                    pes[qi] = pe2

                def transpose_tile(qi):
                    # all 8 [128,128] bf16 transposes of P(qi) land in one psum
                    # tile, evacuated (and converted to fp8) by ONE strided
                    # copy on the Scalar engine.
                    pe2 = pes.pop(qi)
                    ptile = ptiles[qi // 4]
                    tp = psum_tile(f"ptp{l}_{hh}_{qi}")
                    tpr = tp[:, :512].bitcast(BF16)
                    for tk in range(TT):
                        nc.tensor.transpose(tpr[:, tk * P:(tk + 1) * P],
                                            pe2[:, tk * P:(tk + 1) * P], identb)
                    nc.scalar.copy(
                        ptile[:, :, (qi % 4) * P:(qi % 4 + 1) * P],
                        tpr.rearrange("p (tk c) -> p tk c", c=P))

                def pv_half(half, oT):
                    ptile = ptiles[half]
                    # PV for this half: oT[:, m, half*512 : +512]
                    for m in range(DHT):
                        ops_ = psum_tile(f"o{l}_{hh}_{half}_{m}")
                        for j in range(TT // 2):
                            nc.tensor.matmul(
                                ops_[:, :512],
                                lhsT=v4[j][:, :, m * P:(m + 1) * P],
                                rhs=ptile[:, 2 * j:2 * j + 2, :],
                                start=(j == 0), stop=(j == TT // 2 - 1),
                                perf_mode=DR)
                        if m % 2 == 0:
                            nc.vector.tensor_scalar(
                                oT[:, m, half * 512:(half + 1) * 512],
                                ops_[:, :512], OT_EVAC, None, op0=ALU.mult)
                        else:
                            nc.scalar.activation(
                                oT[:, m, half * 512:(half + 1) * 512],
                                ops_[:, :512], AF.Identity, bias=0.0, scale=OT_EVAC)

                # S(q0..q2) first so their ACT/DVE softmax chains run behind
                # the v matmuls; then v; then the remaining S tiles software-
                # pipelined with the P^T transposes (lag 3).
                for qi in range(3):
                    softmax_tile(qi)

                # v token-major: 4 tiles [P(tok), 2, DH] fp8; hT stationary;
                # t-outer, j-inner so each v evacuation fires early.
                v4 = [vpool.tile([P, 2, DH], F8, tag="v", name=f"v{l}_{hh}_{j}")
                      for j in range(TT // 2)]
                vwts = []
                for j in range(DT2):
                    wt = wp5.tile([P, 2, DH], F8, tag="w5", name=f"w5v_{l}_{hh}_{j}")
                    nc.sync.dma_start(wt, _f8(w["wv"].ap()[hh * DT2 + j]))
                    vwts.append(wt)
                pvs = [psum_tile(f"pv{l}_{hh}_{j}") for j in range(TT // 2)]
                for t in range(TT):
                    half = (t % 2) * 512
                    for j in range(DT2):
                        nc.tensor.matmul(
                            pvs[t // 2][:, half:half + DH],
                            lhsT=hT[:, 2 * j:2 * j + 2, t * P:(t + 1) * P],
                            rhs=vwts[j][:, :, :],
                            start=(j == 0), stop=(j == DT2 - 1),
                            perf_mode=DR)
                    nc.scalar.activation(v4[t // 2][:, t % 2, :],
                                         pvs[t // 2][:, half:half + DH],
                                         AF.Identity, bias=0.0, scale=V_EVAC)

                for qi in range(3, TT):
                    softmax_tile(qi)
                    transpose_tile(qi - 3)
                # PV(half0) here covers the softmax-chain tail of q-tiles 5..7
                transpose_tile(TT - 3)
                oT = qko_p.tile([P, DHT, T], F8, tag="qko", name=f"oT{l}_{hh}")
                pv_half(0, oT)
                transpose_tile(TT - 2)
                transpose_tile(TT - 1)
                pv_half(1, oT)

                # o @ wo -> token-major x update, t-outer in two waves of 4
                # token tiles ([128,1024] full-D psum per tile), so each x
                # update (and, on the last head, its LN2 + fT transpose) fires
                # while the next wave's matmuls still feed the PE.
                last = (hh == H - 1)
                if last:
                    fT = big.tile([P, DT, T], F8, tag="hT", name=f"fT{l}")
                owts = {}
                for dh2 in range(2):
                    for j in range(DHT2):
                        wt = wp5.tile([P, 2, 512], F8, tag="w5",
                                      name=f"wo_{l}_{hh}_{dh2}_{j}")
                        nc.sync.dma_start(
                            wt, _f8(w["wo"].ap()[hh * DHT2 + j, :, :,
                                                 dh2 * 512:dh2 * 512 + 512]))
                        owts[(dh2, j)] = wt
                for t in range(TT):
                    px = psum_tile(f"px{l}_{hh}_{t}")
                    for dh2 in range(2):
                        for j in range(DHT2):
                            nc.tensor.matmul(
                                px[:, dh2 * 512:dh2 * 512 + 512],
                                lhsT=oT[:, 2 * j:2 * j + 2, t * P:(t + 1) * P],
                                rhs=owts[(dh2, j)][:, :, :],
                                start=(j == 0), stop=(j == DHT2 - 1),
                                perf_mode=DR)
                    nc.vector.tensor_add(x_tiles[t][:, :], x_tiles[t][:, :], px[:, :])
                    # lag-2 LN2+transpose: the chain of tile t-2 has had two
                    # tiles' worth of PE matmuls to complete, so the PE
                    # doesn't stall on it.
                    if last and t >= 2:
                        _ln_transpose(nc, tc, hpool, small, psum_tile,
                                      x_tiles[t - 2], eps_t, identb, fT, t - 2,
                                      f"f{l}_{t - 2}")
                if last:
                    for t in (TT - 2, TT - 1):
                        _ln_transpose(nc, tc, hpool, small, psum_tile,
                                      x_tiles[t], eps_t, identb, fT, t, f"f{l}_{t}")

            # ---- FFN. After the f2 update of each token tile, the next
            # layer's LN1 + hT transpose (or the final output scale + DMA)
            # is scheduled with a lag of 2 tiles so its LN chain completes
            # behind other PE matmuls.
            if l + 1 < n_layers:
                hT = big.tile([P, DT, T], F8, tag="hT", name=f"hT{l + 1}")
            pending_h = []

            def flush_h(n):
                while pending_h and len(pending_h) > n:
                    tq = pending_h.pop(0)
                    _ln_transpose(nc, tc, hpool, small, psum_tile,
                                  x_tiles[tq], eps_t, identb, hT, tq,
                                  f"h{l + 1}_{tq}")

            for half in range(2):
                toff = half * 512
                # f1 half: feature-major [D, T/2] fp8; w1 stationary;
                # dm-outer, j-inner so each gelu evacuation fires early.
                f1g = ptp.tile([P, DT, 512], F8, tag="pt", name=f"f1g{l}_{half}")
                w1ts = []
                for j in range(DT2):
                    wt = wp10.tile([P, 2, D], F8, tag="w10", name=f"w1_{l}_{half}_{j}")
                    nc.sync.dma_start(wt, _f8(w["w1"].ap()[j]))
                    w1ts.append(wt)
                pfs = [psum_tile(f"pf{l}_{half}_{j}") for j in range(4)]
                for dm in range(DT):
                    pslc = pfs[dm // 2][:, (dm % 2) * 512:(dm % 2) * 512 + 512]
                    for j in range(DT2):
                        nc.tensor.matmul(
                            pslc,
                            lhsT=w1ts[j][:, :, dm * P:(dm + 1) * P],
                            rhs=fT[:, 2 * j:2 * j + 2, toff:toff + 512],
                            start=(j == 0), stop=(j == DT2 - 1),
                            perf_mode=DR)
                    nc.scalar.activation(f1g[:, dm, :], pslc, AF.Gelu,
                                         bias=0.0, scale=GELU_SCALE)
                    if dm == DT - 2:
                        # half-1's f1 matmuls are behind us in the PE queue;
                        # half-0's two pending hT transposes can go now.
                        flush_h(0)
                # f2 half: token-major; f1g stationary; jj-outer, j-inner
                w2ts = []
                for j in range(DT2):
                    wt = wp10.tile([P, 2, D], F8, tag="w10", name=f"w2_{l}_{half}_{j}")
                    nc.sync.dma_start(wt, _f8(w["w2"].ap()[j]))
                    w2ts.append(wt)
                pxs = [psum_tile(f"pg{l}_{half}_{j}") for j in range(4)]
                for jj in range(4):
                    tq = half * 4 + jj
                    for j in range(DT2):
                        for nh in range(2):
                            nc.tensor.matmul(
                                pxs[jj][:, nh * 512:(nh + 1) * 512],
                                lhsT=f1g[:, 2 * j:2 * j + 2, jj * P:(jj + 1) * P],
                                rhs=w2ts[j][:, :, nh * 512:(nh + 1) * 512],
                                start=(j == 0), stop=(j == DT2 - 1),
                                perf_mode=DR)
                    nc.vector.tensor_add(x_tiles[tq][:, :], x_tiles[tq][:, :], pxs[jj][:, :])
                    if l + 1 < n_layers:
                        pending_h.append(tq)
                        flush_h(2)
                    else:
                        nc.scalar.activation(x_tiles[tq][:, :], x_tiles[tq][:, :],
                                             AF.Identity, bias=0.0, scale=1.0 / RS)
                        nc.sync.dma_start(out_d.ap()[tq * P:(tq + 1) * P, :],
                                          x_tiles[tq][:, :])
            if l + 1 < n_layers:
                flush_h(0)

    nc.compile()
    return nc


def _ln_transpose(nc, tc, hpool, small, psum_tile, x_t, eps_t, identb, dstT, t, name):
    """LayerNorm one token tile (output scaled by SH, bf16), transpose it
    into dstT[:, :, t*128:+128] (fp8 conversion at the evacuation copy)."""
    h2 = _ln_tile(nc, tc, hpool, small, x_t, eps_t, name)
    tp = psum_tile(f"tp_{name}")
    tpr = tp[:, :512].bitcast(BF16)
    for d in range(DT):
        nc.tensor.transpose(tpr[:, d * P:(d + 1) * P],
                            h2[:, d * P:(d + 1) * P], identb)
    nc.scalar.copy(dstT[:, :, t * P:(t + 1) * P],
                   tpr.rearrange("p (d c) -> p d c", c=P))


def _ln_tile(nc, tc, hpool, small, x_t, eps_t, name):
    """LayerNorm core SH*(x-mean)*rstd of one [128, D] tile -> bf16 h tile.
    The Sqrt activation computes sqrt(var/256 + eps/256) = sqrt(var+eps)/16,
    so the reciprocal directly yields 16*rstd (= SH fold, no extra op)."""
    stats = small.tile([P, 2, 6], F32, tag="bnst", name=f"st_{name}")
    for g in range(2):
        nc.vector.bn_stats(stats[:, g, :], x_t[:, g * 512:(g + 1) * 512])
    mv = small.tile([P, 2], F32, tag="mv", name=f"mv_{name}")
    nc.vector.bn_aggr(mv, stats)
    std = small.tile([P, 1], F32, tag="std", name=f"sd_{name}")
    nc.scalar.activation(std, mv[:, 1:2], AF.Sqrt, bias=eps_t, scale=1.0 / 256.0)
    rstd = small.tile([P, 1], F32, tag="rstd", name=f"rs_{name}")
    nc.vector.reciprocal(rstd, std)
    h2 = hpool.tile([P, D], BF16, tag="h", name=f"h_{name}")
    nc.vector.tensor_scalar(h2, x_t, mv[:, 0:1], rstd, op0=ALU.subtract, op1=ALU.mult)
    return h2


# ---------------- host side ----------------

def _q8(w, s):
    """Quantize w*s to fp8 e4m3, return as uint8 bytes."""
    return np.asarray(np.asarray(w, np.float32) * s).astype(ml_dtypes.float8_e4m3).view(np.uint8)


def prep_inputs(inputs, n_layers=2):
    """Fold LN gains into weights, quantize to fp8, rearrange for the device.
    Returns (shared_map, per_core_list, use_biases=False)."""
    f = np.float32
    pre_words = np.asarray(inputs["pre_words"])
    img = np.asarray(inputs["img_features"], dtype=f)
    emb = np.ascontiguousarray(np.asarray(inputs["exp_embed"], dtype=f))
    i2v = np.ascontiguousarray(np.asarray(inputs["id2vis"], dtype=f))

    shared = {"emb": emb, "i2v": i2v}
    for l in range(n_layers):
        g1 = np.asarray(inputs["ln1_g"][l], dtype=f)
        b1l = np.asarray(inputs["ln1_b"][l], dtype=f)
        g2 = np.asarray(inputs["ln2_g"][l], dtype=f)
        b2l = np.asarray(inputs["ln2_b"][l], dtype=f)
        wq = np.asarray(inputs["wq"][l], dtype=f) * g1[:, None]
        wk = np.asarray(inputs["wk"][l], dtype=f) * g1[:, None]
        wv = np.asarray(inputs["wv"][l], dtype=f) * g1[:, None]
        wo = np.asarray(inputs["wo"][l], dtype=f)
        w1 = np.asarray(inputs["w1"][l], dtype=f) * g2[:, None]
        w2 = np.asarray(inputs["w2"][l], dtype=f)
        # effective biases must be zero for this kernel (they are, by
        # construction of setup_inputs: zero biases and zero LN betas)
        bq = b1l @ np.asarray(inputs["wq"][l], dtype=f) + np.asarray(inputs["bq"][l], dtype=f)
        bk = b1l @ np.asarray(inputs["wk"][l], dtype=f) + np.asarray(inputs["bk"][l], dtype=f)
        bv = b1l @ np.asarray(inputs["wv"][l], dtype=f) + np.asarray(inputs["bv"][l], dtype=f)
        b1 = b2l @ np.asarray(inputs["w1"][l], dtype=f) + np.asarray(inputs["b1"][l], dtype=f)
        for a in (bq, bk, bv, b1, np.asarray(inputs["bo"][l]), np.asarray(inputs["b2"][l])):
            if np.any(a != 0):
                raise NotImplementedError("nonzero effective biases unsupported")
        # [D, HD] -> [H*DT2, P, 2, DH]; k = j*256 + kp*128 + p
        shared[f"wq{l}"] = np.ascontiguousarray(
            _q8(wq, SW).reshape(DT2, 2, P, H, DH).transpose(3, 0, 2, 1, 4)
            .reshape(H * DT2, P, 2, DH))
        shared[f"wk{l}"] = np.ascontiguousarray(
            _q8(wk, SW).reshape(DT2, 2, P, H, DH).transpose(3, 0, 2, 1, 4)
            .reshape(H * DT2, P, 2, DH))
        shared[f"wv{l}"] = np.ascontiguousarray(
            _q8(wv, SW).reshape(DT2, 2, P, H, DH).transpose(3, 0, 2, 1, 4)
            .reshape(H * DT2, P, 2, DH))
        # [HD, D] -> [H*DHT2, P, 2, D]
        shared[f"wo{l}"] = np.ascontiguousarray(
            _q8(wo, SWO).reshape(H, DHT2, 2, P, D).transpose(0, 1, 3, 2, 4)
            .reshape(H * DHT2, P, 2, D))
        # [D, D] -> [DT2, P, 2, D]
        shared[f"w1{l}"] = np.ascontiguousarray(
            _q8(w1, SW).reshape(DT2, 2, P, D).transpose(0, 2, 1, 3))
        shared[f"w2{l}"] = np.ascontiguousarray(
            _q8(w2, SW2).reshape(DT2, 2, P, D).transpose(0, 2, 1, 3))

    per_core = []
    for b in range(B):
        idx = np.ascontiguousarray(
            pre_words[b].astype(np.int32).reshape(TT, P).T)
        per_core.append({"idx": idx,
                         "img": np.ascontiguousarray(img[b] * RS)})
    return shared, per_core, False


def make_in_maps(shared, per_core, use_biases=False, n_layers=2):
    keys = ["emb", "i2v"]
    for l in range(n_layers):
        keys += [f"wq{l}", f"wk{l}", f"wv{l}", f"wo{l}", f"w1{l}", f"w2{l}"]
    maps = []
    for b in range(B):
        m = {k: shared[k] for k in keys}
        m.update(per_core[b])
        maps.append(m)
    return maps


# ---------------- public entry point ----------------

_CACHE = {}


def _get_nc(n_layers=2, use_biases=False):
    key = n_layers
    if key not in _CACHE:
        _CACHE[key] = build_nc(n_layers=n_layers)
    return _CACHE[key]


def kernel(**inputs):
    shared, per_core, use_biases = prep_inputs(inputs, n_layers=2)
    nc = _get_nc(2, use_biases)
    in_maps = make_in_maps(shared, per_core, use_biases, n_layers=2)
    res = run_bass_kernel_spmd(nc, in_maps, list(range(8)))
    out = np.stack([res.results[i]["out"] for i in range(8)]).astype(np.float32)
    return out
